# revision 12
# baseline (speedup 1.0000x reference)
"""EngramLayer Trainium2 kernel v4 (8-core SPMD, Bass/Tile).

v2: batched gather (1 indirect DMA/tile), fp8 DoubleRow k-proj, bf16 output.
v3: v evacuated to SBUF right after proj (PSUM released without gate dep),
    host-precomputed sum(h^2), retuned PSUM pools.
v4: software-pipelined emission — per-engine queues are FIFO, so tile i's
    gate-dependent back half (yn transposes, conv, silu, store) is emitted
    AFTER tile i+1's front half; the gate-chain latency then overlaps next
    tile's projections instead of head-of-line blocking the PE.  Also
    interleaves k/v projection groups (k_q, v_q) so PSUM consumers drain
    while the other projection streams.
"""

import math

import numpy as np
import ml_dtypes

import concourse.bass as bass
import concourse.bacc as bacc
import concourse.mybir as mybir
import concourse.tile as tile
from concourse import bass_utils

F32 = mybir.dt.float32
BF16 = mybir.dt.bfloat16
FP8 = mybir.dt.float8e4
I32 = mybir.dt.int32
AF = mybir.ActivationFunctionType
OP = mybir.AluOpType
PM = mybir.MatmulPerfMode

P = 128
B, T, D = 4, 4096, 2048
DM, H, DH = 1024, 16, 64
TABLE = 131072
NCORES = 8
TOK_OUT = (B * T) // NCORES          # 2048 output tokens per core
NT = TOK_OUT // P + 1                # 17 tiles (tile 0 = halo)
NM = DM // P                         # 8 m-tiles
ND = D // P                          # 16 d-tiles
NQ = 4                               # 512-wide d quarters
EPS_QK = float(np.finfo(np.float32).eps)
EPS_CONV = 1e-5
KK, DIL = 4, 2
CV = 256.0                           # emb table host-scale (v = CV * v_true)
CW = 128.0                           # key_W host-scale (k = CV*CW * k_true)

_CACHE = {}
Y_TRANSPOSED = True


def _rsqrt(nc, pool, x, tag):
    """rsqrt on a [128,1] fp32 AP via Quake init + 2 Newton iterations."""
    it_ = pool.tile([P, 1], I32, tag=f"{tag}_i")
    nc.vector.tensor_scalar(out=it_[:], in0=x.bitcast(I32), scalar1=1,
                            scalar2=None, op0=OP.logical_shift_right)
    nc.vector.tensor_scalar(out=it_[:], in0=it_[:], scalar1=-1, scalar2=None,
                            op0=OP.bitwise_xor)
    nc.vector.tensor_scalar(out=it_[:], in0=it_[:], scalar1=0x5F3759DF + 1,
                            scalar2=None, op0=OP.add)
    y = pool.tile([P, 1], F32, tag=f"{tag}_y")
    t1 = pool.tile([P, 1], F32, tag=f"{tag}_t")
    src = it_[:].bitcast(F32)
    for _ in range(2):
        nc.vector.tensor_tensor(out=t1[:], in0=x, in1=src, op=OP.mult)
        nc.vector.tensor_tensor(out=t1[:], in0=t1[:], in1=src, op=OP.mult)
        nc.vector.tensor_scalar(out=t1[:], in0=t1[:], scalar1=-0.5,
                                scalar2=1.5, op0=OP.mult, op1=OP.add)
        nc.vector.tensor_tensor(out=y[:], in0=src, in1=t1[:], op=OP.mult)
        src = y[:]
    return y


def build(silu_via_sigmoid=False):
    nc = bacc.Bacc(None, target_bir_lowering=False)
    ntok = NT * P

    h_in = nc.dram_tensor("h", [ntok, D], BF16, kind="ExternalInput")
    hhp = nc.dram_tensor("hhp", [P, NT], F32, kind="ExternalInput")
    hidx = nc.dram_tensor("hidx", [ntok, H], I32, kind="ExternalInput")
    tbl = nc.dram_tensor("tbl", [H * TABLE, DH], BF16, kind="ExternalInput")
    kwt = nc.dram_tensor("kwt", [NM, P, D], FP8, kind="ExternalInput")
    vwt = nc.dram_tensor("vwt", [NM, P, D], BF16, kind="ExternalInput")
    cdg = nc.dram_tensor("cdg", [KK * ND, P, P], BF16, kind="ExternalInput")
    idn = nc.dram_tensor("idn", [P, P], BF16, kind="ExternalInput")
    msk = nc.dram_tensor("msk", [P, 1], F32, kind="ExternalInput")
    # d-major output: y^T [D, tokens]; host transposes back
    y_out = nc.dram_tensor("y", [D, ntok - P], BF16, kind="ExternalOutput")

    with tile.TileContext(nc) as tc:
        with (
            tc.tile_pool(name="const", bufs=1) as cp,
            tc.tile_pool(name="io", bufs=3) as io,
            tc.tile_pool(name="work", bufs=3) as wk,
            tc.tile_pool(name="blk", bufs=2) as bp,
            tc.tile_pool(name="sblk", bufs=1) as sp,
            tc.tile_pool(name="gvp", bufs=6) as gp,
            tc.tile_pool(name="stat", bufs=3) as st,
            tc.tile_pool(name="pkk", bufs=2, space="PSUM") as pkk,
            tc.tile_pool(name="pkv", bufs=2, space="PSUM") as pkv,
            tc.tile_pool(name="ptr", bufs=2, space="PSUM") as ptr,
            tc.tile_pool(name="pc", bufs=2, space="PSUM") as pcp,
        ):
            # ---- resident constants ----
            idn_sb = cp.tile([P, P], BF16)
            nc.sync.dma_start(idn_sb[:], idn[:])
            msk_sb = cp.tile([P, 1], F32)
            nc.sync.dma_start(msk_sb[:], msk[:])
            hh_sb = cp.tile([P, NT], F32)
            nc.sync.dma_start(hh_sb[:], hhp[:])
            one_row = cp.tile([1, P], BF16)
            nc.vector.memset(one_row[:], 1.0)
            # weights on the scalar HWDGE ring: io loads don't queue behind them
            kwt_sb = cp.tile([P, NM, D], FP8)
            vwt_sb = cp.tile([P, NM, D], BF16)
            for m in range(NM):
                nc.scalar.dma_start(kwt_sb[:, m, :], kwt[m])
                nc.scalar.dma_start(vwt_sb[:, m, :], vwt[m])
            cdg_sb = cp.tile([P, KK * ND, P], BF16)
            nc.scalar.dma_start(cdg_sb[:], cdg[:].rearrange("i p q -> p i q"))

            state = {}   # per-tile tiles handed from front(i) to back(i)
            prev = {}

            def front(i):
                it_ = io.tile([P, H], I32, tag="idx")
                nc.sync.dma_start(it_[:], hidx[i * P:(i + 1) * P, :])
                h_sb = io.tile([P, D], BF16, tag="h")
                nc.sync.dma_start(h_sb[:], h_in[i * P:(i + 1) * P, :])

                e_sb = io.tile([P, DM], BF16, tag="e")
                # HW indirect DMA supports only 1 index/partition -> 16 DMAs
                for hh in range(H):
                    nc.gpsimd.indirect_dma_start(
                        out=e_sb[:, hh * DH:(hh + 1) * DH], out_offset=None,
                        in_=tbl[:],
                        in_offset=bass.IndirectOffsetOnAxis(
                            ap=it_[:, hh:hh + 1], axis=0))

                pt_e = ptr.tile([P, DM], BF16, tag="tr")
                for m in range(NM):
                    nc.tensor.transpose(pt_e[:, m * P:(m + 1) * P],
                                        e_sb[:, m * P:(m + 1) * P], idn_sb[:])
                eb = wk.tile([P, NM, P], BF16, tag="eb")
                nc.vector.tensor_copy(
                    eb[:], pt_e[:].rearrange("p (m t) -> p m t", m=NM))
                e8 = wk.tile([P, NM, P], FP8, tag="e8")
                nc.scalar.copy(
                    e8[:], pt_e[:].rearrange("p (m t) -> p m t", m=NM))

                acc_hk = st.tile([P, NQ], F32, tag="acc_hk")
                acc_kk = st.tile([P, NQ], F32, tag="acc_kk")
                acc_vv = st.tile([P, NQ], F32, tag="acc_vv")
                scr = wk.tile([P, 512], BF16, tag="scr")
                v_sb = wk.tile([P, D], BF16, tag="vsb")

                # interleaved projections: [k_q, v_q] x 4; consumers drain a
                # PSUM bank while the other projection streams
                for q in range(NQ):
                    sl = slice(q * 512, (q + 1) * 512)
                    kq = pkk.tile([P, 512], F32, tag="kq")
                    for mi in range(NM // 2):
                        nc.tensor.matmul(
                            kq[:],
                            e8[:, 2 * mi:2 * mi + 2, :],
                            kwt_sb[:, 2 * mi:2 * mi + 2, sl],
                            start=(mi == 0), stop=(mi == NM // 2 - 1),
                            perf_mode=PM.DoubleRow)
                    vq = pkv.tile([P, 512], F32, tag="vq")
                    for m in range(NM):
                        nc.tensor.matmul(vq[:], eb[:, m, :], vwt_sb[:, m, sl],
                                         start=(m == 0), stop=(m == NM - 1))
                    nc.vector.scalar_tensor_tensor(
                        out=scr[:], in0=h_sb[:, sl], scalar=1.0, in1=kq[:],
                        op0=OP.mult, op1=OP.mult, accum_out=acc_hk[:, q:q + 1])
                    k2scr = wk.tile([P, 512], BF16, tag="k2scr")
                    nc.scalar.activation(k2scr[:], kq[:], AF.Square,
                                         accum_out=acc_kk[:, q:q + 1])
                    v2scr = wk.tile([P, 512], BF16, tag="v2scr")
                    nc.scalar.activation(v2scr[:], vq[:], AF.Square,
                                         accum_out=acc_vv[:, q:q + 1])
                    if q % 2 == 0:
                        nc.vector.tensor_copy(v_sb[:, sl], vq[:])
                    else:
                        nc.scalar.copy(v_sb[:, sl], vq[:])

                # ---- gate chain on [128,1] ----
                s_hk = st.tile([P, 1], F32, tag="s_hk")
                s_kk = st.tile([P, 1], F32, tag="s_kk")
                s_vv = st.tile([P, 1], F32, tag="s_vv")
                nc.vector.reduce_sum(s_hk[:], acc_hk[:], axis=mybir.AxisListType.X)
                nc.vector.reduce_sum(s_kk[:], acc_kk[:], axis=mybir.AxisListType.X)
                nc.vector.reduce_sum(s_vv[:], acc_vv[:], axis=mybir.AxisListType.X)

                msq = st.tile([P, 1], F32, tag="msq")
                msk_ = st.tile([P, 1], F32, tag="msk_")
                pp = st.tile([P, 1], F32, tag="pp")
                nc.vector.tensor_scalar(out=msq[:], in0=hh_sb[:, i:i + 1],
                                        scalar1=1.0 / D, scalar2=EPS_QK,
                                        op0=OP.mult, op1=OP.add)
                nc.vector.tensor_scalar(out=msk_[:], in0=s_kk[:], scalar1=1.0 / D,
                                        scalar2=EPS_QK, op0=OP.mult, op1=OP.add)
                nc.vector.scalar_tensor_tensor(
                    out=pp[:], in0=msq[:], scalar=float(D), in1=msk_[:],
                    op0=OP.mult, op1=OP.mult)
                r1 = _rsqrt(nc, st, pp[:], "r1")
                dot = st.tile([P, 1], F32, tag="dot")
                nc.vector.tensor_tensor(out=dot[:], in0=s_hk[:], in1=r1[:], op=OP.mult)
                ad = st.tile([P, 1], F32, tag="ad")
                nc.vector.scalar_tensor_tensor(
                    out=ad[:], in0=dot[:], scalar=-1.0, in1=dot[:],
                    op0=OP.mult, op1=OP.max)
                nc.vector.tensor_scalar(out=ad[:], in0=ad[:], scalar1=1e-6,
                                        scalar2=None, op0=OP.max)
                r2 = _rsqrt(nc, st, ad[:], "r2")
                u = st.tile([P, 1], F32, tag="u")
                nc.vector.tensor_tensor(out=u[:], in0=dot[:], in1=r2[:], op=OP.mult)
                th = st.tile([P, 1], F32, tag="th")
                nc.scalar.activation(th[:], u[:], AF.Tanh, scale=0.5)
                gate = st.tile([P, 1], F32, tag="gate")
                nc.vector.tensor_scalar(out=gate[:], in0=th[:], scalar1=0.5 / CV,
                                        scalar2=0.5 / CV, op0=OP.mult, op1=OP.add)
                if i == 0:
                    nc.vector.tensor_tensor(out=gate[:], in0=gate[:],
                                            in1=msk_sb[:], op=OP.mult)
                gg = st.tile([P, 1], F32, tag="gg")
                nc.vector.tensor_tensor(out=gg[:], in0=gate[:], in1=gate[:], op=OP.mult)
                mv = st.tile([P, 1], F32, tag="mv")
                nc.vector.tensor_scalar(out=mv[:], in0=s_vv[:], scalar1=1.0 / D,
                                        scalar2=None, op0=OP.mult)
                mc = st.tile([P, 1], F32, tag="mc")
                nc.vector.scalar_tensor_tensor(
                    out=mc[:], in0=gg[:], scalar=EPS_CONV, in1=mv[:],
                    op0=OP.bypass, op1=OP.mult)
                nc.vector.tensor_scalar(out=mc[:], in0=mc[:], scalar1=EPS_CONV,
                                        scalar2=None, op0=OP.add)
                rc = _rsqrt(nc, st, mc[:], "rc")
                # gr = gate*rc (yn = v*gr), irc = 1/rc = rc*mc (y = yn*irc+silu)
                gr = st.tile([P, 1], F32, tag="gr")
                nc.vector.tensor_tensor(out=gr[:], in0=gate[:], in1=rc[:],
                                        op=OP.mult)
                irc = gp.tile([P, 1], BF16, tag="irc")
                nc.vector.tensor_tensor(out=irc[:], in0=rc[:], in1=mc[:],
                                        op=OP.mult)

                state[i] = dict(v_sb=v_sb, gr=gr, irc=irc)

            # conv runs on 512-token blocks of 4 tiles: fewer, wider matmuls
            BLK = 4

            def back(i):
                """yn for tile i, transposed into its block's ynb buffer."""
                s = state.pop(i)
                v_sb, gr = s["v_sb"], s["gr"]
                state[("irc", i)] = s["irc"]

                yn = wk.tile([P, D], BF16, tag="yn")
                nc.vector.tensor_scalar(out=yn[:], in0=v_sb[:], scalar1=gr[:],
                                        scalar2=None, op0=OP.mult)

                j = i % BLK
                if j == 0:
                    ynb = bp.tile([P, ND, BLK * P + 8], BF16, tag="ynb")
                    if prev.get("ynb") is not None:
                        pb, pn = prev["ynb"], prev["ynb_ntok"]
                        nc.vector.tensor_copy(ynb[:, :, 0:8],
                                              pb[:, :, pn:pn + 8])
                    else:
                        nc.vector.memset(ynb[:, :, 0:8], 0.0)
                    prev["ynb"] = ynb
                else:
                    ynb = prev["ynb"]

                for half in range(2):
                    pt_h = ptr.tile([P, 1024], BF16, tag="tr")
                    for jj in range(8):
                        dt = half * 8 + jj
                        nc.tensor.transpose(pt_h[:, jj * P:(jj + 1) * P],
                                            yn[:, dt * P:(dt + 1) * P], idn_sb[:])
                    nc.vector.tensor_copy(
                        ynb[:, half * 8:(half + 1) * 8,
                            8 + j * P:8 + (j + 1) * P],
                        pt_h[:].rearrange("p (d t) -> p d t", d=8))

            def blockend(i0, ntile):
                """conv + silu + transpose-back + add + store for tiles
                i0..i0+ntile-1 (ynb holds their yn, 8-col halo at front)."""
                ynb = prev["ynb"]
                prev["ynb_ntok"] = ntile * P
                ntok_b = ntile * P

                silu_blk = sp.tile([P, ND, BLK * P], BF16, tag="silu_blk")
                for dt in range(ND):
                    yc = pcp.tile([P, 512], F32, tag="yc")
                    for k in range(KK):
                        off = 2 + 2 * k
                        nc.tensor.matmul(
                            yc[:, 0:ntok_b],
                            cdg_sb[:, k * ND + dt, :],
                            ynb[:, dt, off:off + ntok_b],
                            start=(k == 0), stop=(k == KK - 1))
                    if silu_via_sigmoid:
                        sg = sp.tile([P, 512], F32, tag="sgm")
                        nc.scalar.activation(sg[:, 0:ntok_b], yc[:, 0:ntok_b],
                                             AF.Sigmoid)
                        nc.vector.tensor_mul(silu_blk[:, dt, 0:ntok_b],
                                             sg[:, 0:ntok_b], yc[:, 0:ntok_b])
                    else:
                        nc.scalar.activation(silu_blk[:, dt, 0:ntok_b],
                                             yc[:, 0:ntok_b], AF.Silu)

                # broadcast row of irc over partitions: transpose each tile's
                # [128,1] column into one [1, ntok_b] row, then rank-1 matmul
                pt_row = ptr.tile([P, 1024], BF16, tag="tr")
                for j in range(ntile):
                    irc = state.pop(("irc", i0 + j))
                    nc.tensor.transpose(pt_row[0:1, j * P:(j + 1) * P],
                                        irc[:], idn_sb[:])
                row_sb = wk.tile([1, BLK * P], BF16, tag="row")
                nc.vector.tensor_copy(row_sb[0:1, 0:ntok_b],
                                      pt_row[0:1, 0:ntok_b])
                bc_ps = pcp.tile([P, 512], F32, tag="yc")
                nc.tensor.matmul(bc_ps[:, 0:ntok_b], one_row[:],
                                 row_sb[0:1, 0:ntok_b], start=True, stop=True)
                bc_sb = wk.tile([P, BLK * P], BF16, tag="bc")
                nc.vector.tensor_copy(bc_sb[:, 0:ntok_b], bc_ps[:, 0:ntok_b])

                # y^T[dt] = yn^T[dt]*irc_bc + silu (gv^T = yn^T/rc)
                src0 = P if i0 == 0 else 0
                dst0 = 0 if i0 == 0 else (i0 - 1) * P
                for dt in range(ND):
                    tmp = wk.tile([P, 512], BF16, tag="scr2")
                    nc.vector.tensor_mul(tmp[:, 0:ntok_b],
                                         ynb[:, dt, 8:8 + ntok_b],
                                         bc_sb[:, 0:ntok_b])
                    nc.vector.tensor_add(ynb[:, dt, 0:ntok_b],
                                         tmp[:, 0:ntok_b],
                                         silu_blk[:, dt, 0:ntok_b])
                    if ntok_b > src0:
                        nc.sync.dma_start(
                            y_out[dt * P:(dt + 1) * P,
                                  dst0:dst0 + ntok_b - src0],
                            ynb[:, dt, src0:ntok_b])

            # ---- skewed pipeline: front(i+1) runs ahead of back(i) ----
            for i in range(NT + 1):
                if i < NT:
                    front(i)
                if i >= 1:
                    back(i - 1)
                    if (i - 1) % BLK == BLK - 1 or i - 1 == NT - 1:
                        i0 = ((i - 1) // BLK) * BLK
                        blockend(i0, i - i0)

    nc.compile()
    return nc


def _host_prep(inputs):
    """Shared (per-run) host-side constant prep."""
    bf = ml_dtypes.bfloat16
    f8 = ml_dtypes.float8_e4m3
    tbl = (np.ascontiguousarray(inputs["emb_table"]) * CV).astype(bf)
    kwt = (np.ascontiguousarray(inputs["key_W"].T.reshape(NM, P, D)) * CW
           ).astype(f8)
    vwt = np.ascontiguousarray(inputs["value_W"].T.reshape(NM, P, D)).astype(bf)
    cw = np.asarray(inputs["conv_w"])  # [D, 1, K]
    cdg = np.zeros((KK * ND, P, P), dtype=bf)
    for k in range(KK):
        for dt in range(ND):
            np.fill_diagonal(cdg[k * ND + dt],
                             cw[dt * P:(dt + 1) * P, 0, k].astype(bf))
    idn = np.eye(P, dtype=bf)
    flat_h = np.asarray(inputs["hidden_states"]).reshape(B * T, D)
    flat_hh = np.einsum("td,td->t", flat_h, flat_h, dtype=np.float32)
    flat_ids = np.asarray(inputs["hash_ids"]).reshape(B * T, H).astype(np.int64)
    flat_ids = (flat_ids + (np.arange(H, dtype=np.int64) * TABLE)[None, :])
    flat_ids = flat_ids.astype(np.int32)
    return tbl, kwt, vwt, cdg, idn, flat_h, flat_hh, flat_ids


def make_in_maps(inputs):
    bf = ml_dtypes.bfloat16
    tbl, kwt, vwt, cdg, idn, flat_h, flat_hh, flat_ids = _host_prep(inputs)

    in_maps = []
    for c in range(NCORES):
        t0 = c * TOK_OUT
        h_c = np.zeros((NT * P, D), dtype=bf)
        hh_c = np.zeros((NT * P,), dtype=np.float32)
        ids_c = np.zeros((NT * P, H), dtype=np.int32)
        valid_halo = (t0 % T) != 0
        lo = t0 - P
        if valid_halo:
            h_c[:] = flat_h[lo:t0 + TOK_OUT].astype(bf)
            hh_c[:] = flat_hh[lo:t0 + TOK_OUT]
            ids_c[:] = flat_ids[lo:t0 + TOK_OUT]
        else:
            h_c[P:] = flat_h[t0:t0 + TOK_OUT].astype(bf)
            hh_c[P:] = flat_hh[t0:t0 + TOK_OUT]
            ids_c[P:] = flat_ids[t0:t0 + TOK_OUT]
        hhp = np.ascontiguousarray(hh_c.reshape(NT, P).T)
        msk = np.full((P, 1), 1.0 if valid_halo else 0.0, dtype=np.float32)
        in_maps.append(dict(h=h_c, hhp=hhp, hidx=ids_c, tbl=tbl, kwt=kwt,
                            vwt=vwt, cdg=cdg, idn=idn, msk=msk))
    return in_maps


def kernel(**inputs):
    if "nc" not in _CACHE:
        _CACHE["nc"] = build()
    nc = _CACHE["nc"]
    in_maps = make_in_maps(inputs)

    res = bass_utils.run_bass_kernel_spmd(nc, in_maps, core_ids=list(range(NCORES)))
    # outputs are d-major [D, tok_out] per core -> transpose and concat
    y = np.concatenate([np.asarray(res.results[c]["y"], dtype=np.float32).T
                        for c in range(NCORES)], axis=0)
    return y.reshape(B, T, D)


if __name__ == "__main__":
    build()
    print("build OK")
